# revision 28
# baseline (speedup 1.0000x reference)
"""Bahdanau attention TRN2 Bass kernel (SPMD over 8 NeuronCores).

Problem: S=4096, N=32, H=512 (fp32)
    wh   = hidden @ W_w.T + W_b                      # [N, H]
    ue   = einsum('snh,kh->snk', enc, U_w) + U_b     # [S, N, H]
    en   = tanh(wh + ue)                             # [S, N, H]
    sc   = einsum('snh,oh->sno', en, v_w) + v_b      # [S, N, 1]
    aw   = softmax(sc, axis=0)                       # [S, N, 1]
    ctx  = sum_s aw * enc                            # [N, H]
    returns (ctx, aw)

Sharding: data-parallel over N (4 examples per core), weights replicated.
No cross-core communication.

Single pass over encoder_outputs (the 256 MiB tensor):
  - host feeds enc transposed per-example: encT[n, h, s] (so the TensorE
    contraction dim h lands on SBUF partitions with fast contiguous DMA)
  - ue^T tile [k=128, s=512] = sum_hc U^T_block.T @ encT_tile   (f32r matmuls)
  - energy = tanh(ue^T + (wh+U_b) per-partition bias)           (ACT, fused)
  - score row = sum_k v128.T @ energy  (v replicated across 128 cols so every
    PSUM partition carries the same score row)
  - w = exp(score)  (no max-subtraction needed: |score| <= ||v||_1 ~ 11, and
    exp of that is comfortably inside fp32; softmax is shift-invariant so v_b
    is dropped entirely)  + accum_out gives the Z partial for free
  - ctx partial via one fused DVE tensor_tensor_reduce per (h-chunk, s-block):
    (encT_tile * w) reduced over s -> [128, 1]
  - finals: Z = sum of partials, ctx *= 1/Z, attw row *= 1/Z, DMA out.
"""

import os
from contextlib import ExitStack

import numpy as np

S = 4096
N = 32
H = 512
NCORES = 8
NL = N // NCORES  # 4 examples per core
P = 128
HC = H // P  # 4 h-chunks
KC = H // P  # 4 k-chunks
SBW = 512  # s-block width
SB = S // SBW  # 8 s-blocks

_CACHE = {}


def _build_nc(nl=NL, sb_count=SB):
    import concourse.mybir as mybir
    from concourse.bacc import Bacc
    from concourse.tile import TileContext

    F32 = mybir.dt.float32
    F32R = mybir.dt.float32r
    BF16 = mybir.dt.bfloat16
    AF = mybir.ActivationFunctionType
    ALU = mybir.AluOpType
    AX = mybir.AxisListType

    nc = Bacc(trn_type="TRN2")

    encT = nc.declare_dram_parameter("encT", [NL, H, S], F32R, isOutput=False)
    hidT = nc.declare_dram_parameter("hidT", [H, NL], F32R, isOutput=False)
    WT = nc.declare_dram_parameter("WT", [H, H], F32R, isOutput=False)
    UT = nc.declare_dram_parameter("UT", [H, H], F32R, isOutput=False)
    wb2 = nc.declare_dram_parameter("wb2", [P, KC], F32, isOutput=False)
    ub2 = nc.declare_dram_parameter("ub2", [P, KC], F32, isOutput=False)
    v2 = nc.declare_dram_parameter("v2", [P, KC], F32, isOutput=False)
    ctx_out = nc.declare_dram_parameter("ctx", [NL, H], F32, isOutput=True)
    attw_out = nc.declare_dram_parameter("attw", [NL, S], F32, isOutput=True)

    with TileContext(nc) as tc, ExitStack() as ctx:
        consts = ctx.enter_context(tc.tile_pool(name="consts", bufs=1))
        enc_pool = ctx.enter_context(tc.tile_pool(name="enc", bufs=4))
        en_pool = ctx.enter_context(tc.tile_pool(name="en", bufs=4))
        w_pool = ctx.enter_context(tc.tile_pool(name="w", bufs=3))
        fin_pool = ctx.enter_context(tc.tile_pool(name="fin", bufs=2))
        scr_pool = ctx.enter_context(tc.tile_pool(name="scr", bufs=2))
        psum_ue = ctx.enter_context(tc.tile_pool(name="psue", bufs=4, space="PSUM"))
        psum_sc = ctx.enter_context(tc.tile_pool(name="pssc", bufs=3, space="PSUM"))

        # ---- constants ----
        ut_sb = consts.tile([P, HC, H], F32R)
        UTr = UT.rearrange("(c p) k -> p c k", p=P)
        for hc in range(HC):
            nc.sync.dma_start(out=ut_sb[:, hc, :], in_=UTr[:, hc, :])
        wt_sb = consts.tile([P, HC, H], F32R)
        nc.sync.dma_start(out=wt_sb, in_=WT.rearrange("(c p) k -> p c k", p=P))
        hid_sb = consts.tile([P, HC, NL], F32R)
        nc.sync.dma_start(out=hid_sb, in_=hidT.rearrange("(c p) n -> p c n", p=P))
        wb2_sb = consts.tile([P, KC], F32)
        nc.sync.dma_start(out=wb2_sb, in_=wb2[:])
        ub2_sb = consts.tile([P, KC], F32)
        nc.sync.dma_start(out=ub2_sb, in_=ub2[:])
        v2_sb = consts.tile([P, KC], F32)
        nc.sync.dma_start(out=v2_sb, in_=v2[:])

        bias2 = consts.tile([P, KC], F32)
        nc.vector.tensor_add(bias2, wb2_sb, ub2_sb)

        ones128 = consts.tile([P, P], F32)
        nc.vector.memset(ones128, 1.0)
        v128 = consts.tile([P, KC, P], F32R)
        for kc in range(KC):
            nc.vector.tensor_scalar_mul(v128[:, kc, :], ones128, v2_sb[:, kc : kc + 1])

        # ---- wh^T = W @ hidden^T, bias-folded: whb[k, n] = wh^T + W_b + U_b ----
        whb = consts.tile([P, KC, NL], F32)
        for kc in range(KC):
            pw = psum_ue.tile([P, NL], F32, tag="ue")
            for hc in range(HC):
                nc.tensor.matmul(
                    pw,
                    lhsT=wt_sb[:, hc, kc * P : (kc + 1) * P],
                    rhs=hid_sb[:, hc, :],
                    start=(hc == 0),
                    stop=(hc == HC - 1),
                )
            nc.vector.tensor_scalar_add(whb[:, kc, :], pw, bias2[:, kc : kc + 1])

        # ---- accumulators ----
        zp_n = [consts.tile([P, SB], F32, name=f"zp{i}") for i in range(NL)]
        ctxp_n = [consts.tile([P, HC, SB], F32, name=f"ctxp{i}") for i in range(NL)]
        # per-example attention rows: separate [1, S] tiles (partition 0), fed
        # from row 0 of w (all rows of w are identical) — keeps examples
        # dependency-independent and partition-legal
        attw_n = [consts.tile([1, S], F32, name=f"attw{i}") for i in range(NL)]

        # ---- main loop ----
        for n in range(nl):
            encTn = encT[n].rearrange("(c p) s -> p c s", p=P)
            for sb in range(sb_count):
                # one DMA brings all 4 h-chunks of this s-block: [p, hc, s]
                enc4 = enc_pool.tile([P, HC, SBW], F32R, tag="enc")
                nc.sync.dma_start(
                    out=enc4, in_=encTn[:, :, sb * SBW : (sb + 1) * SBW]
                )
                psc = psum_sc.tile([P, SBW], F32, tag="sc")
                for kc in range(KC):
                    pue = psum_ue.tile([P, SBW], F32, tag="ue")
                    for hc in range(HC):
                        nc.tensor.matmul(
                            pue,
                            lhsT=ut_sb[:, hc, kc * P : (kc + 1) * P],
                            rhs=enc4[:, hc, :],
                            start=(hc == 0),
                            stop=(hc == HC - 1),
                        )
                    en = en_pool.tile([P, SBW], F32R, tag="en")
                    nc.scalar.activation(
                        out=en,
                        in_=pue,
                        func=AF.Tanh,
                        bias=whb[:, kc, n : n + 1],
                        scale=1.0,
                    )
                    nc.tensor.matmul(
                        psc,
                        lhsT=v128[:, kc, :],
                        rhs=en,
                        start=(kc == 0),
                        stop=(kc == KC - 1),
                    )
                w = w_pool.tile([P, SBW], F32, tag="w")
                nc.scalar.activation(
                    out=w,
                    in_=psc,
                    func=AF.Exp,
                    accum_out=zp_n[n][:, sb : sb + 1],
                )
                # stash the (unnormalized) attention row for this block; row n of
                # w is identical to every other row, and lives on partition n so
                # a plain DVE copy into attw_all's row n is partition-legal.
                nc.gpsimd.tensor_copy(
                    attw_n[n][0:1, sb * SBW : (sb + 1) * SBW], w[0:1, :]
                )
                scr = scr_pool.tile([P, HC, SBW], F32, tag="scr")
                for hc in range(HC):
                    nc.vector.affine_mul_reduce(
                        out=scr[:, hc, :],
                        accum_out=ctxp_n[n][:, hc, sb : sb + 1],
                        in0=enc4[:, hc, :].bitcast(F32),
                        in1=w,
                        scale=1.0,
                        bias=0.0,
                    )

            # ---- per-example finals (overlap with next n's main loop) ----
            z1 = fin_pool.tile([P, 1], F32, tag="z1")
            nc.vector.reduce_sum(out=z1, in_=zp_n[n], axis=AX.X)
            rz = fin_pool.tile([P, 1], F32, tag="rz")
            nc.vector.reciprocal(out=rz, in_=z1)
            for hc in range(HC):
                c1 = fin_pool.tile([P, 1], F32, tag="c1")
                nc.vector.reduce_sum(out=c1, in_=ctxp_n[n][:, hc, :], axis=AX.X)
                c2 = fin_pool.tile([P, 1], F32, tag="c2")
                nc.gpsimd.tensor_mul(c2, c1, rz)
                nc.sync.dma_start(out=ctx_out[n, hc * P : (hc + 1) * P], in_=c2[:, 0])
            nc.gpsimd.tensor_scalar_mul(attw_n[n], attw_n[n], rz[0:1, 0:1])
            nc.sync.dma_start(out=attw_out[n : n + 1, :], in_=attw_n[n])

    nc.finalize()
    return nc


def _get_nc():
    if "nc" not in _CACHE:
        _CACHE["nc"] = _build_nc()
    return _CACHE["nc"]


def _prep_inputs(hidden, encoder_outputs, W_w, W_b, U_w, U_b, v_w, v_b):
    f = lambda x: np.ascontiguousarray(np.asarray(x, dtype=np.float32))
    hidden = f(hidden)
    W_w, W_b, U_w, U_b, v_w = f(W_w), f(W_b), f(U_w), f(U_b), f(v_w)
    enc = np.asarray(encoder_outputs, dtype=np.float32)

    encT_all = np.ascontiguousarray(enc.transpose(1, 2, 0))  # [N, H, S]
    hidT_all = np.ascontiguousarray(hidden.T)  # [H, N]
    WT = np.ascontiguousarray(W_w.T)
    UT = np.ascontiguousarray(U_w.T)
    wb2 = np.ascontiguousarray(W_b.reshape(KC, P).T)
    ub2 = np.ascontiguousarray(U_b.reshape(KC, P).T)
    v2 = np.ascontiguousarray(v_w.reshape(KC, P).T)

    in_maps = []
    for c in range(NCORES):
        n0 = c * NL
        in_maps.append(
            {
                "encT": encT_all[n0 : n0 + NL],
                "hidT": np.ascontiguousarray(hidT_all[:, n0 : n0 + NL]),
                "WT": WT,
                "UT": UT,
                "wb2": wb2,
                "ub2": ub2,
                "v2": v2,
            }
        )
    return in_maps


def run(trace=False, **inputs):
    """Run the SPMD kernel; returns ((context, attention_weights), raw results)."""
    from concourse.bass_utils import run_bass_kernel_spmd

    in_maps = _prep_inputs(**inputs)
    nc = _get_nc()
    res = run_bass_kernel_spmd(nc, in_maps, core_ids=list(range(NCORES)), trace=trace)
    ctxs = np.concatenate([res.results[i]["ctx"] for i in range(NCORES)], axis=0)
    attws = np.concatenate([res.results[i]["attw"] for i in range(NCORES)], axis=0)
    attention_weights = np.ascontiguousarray(attws.T)[:, :, None]
    return (ctxs, attention_weights), res


def kernel(**inputs):
    out, _ = run(trace=False, **inputs)
    return out


if __name__ == "__main__":
    # quick smoke: build only
    nc = _get_nc()
    print("built ok, instructions:", len(nc.inst_map))


# revision 30
# speedup vs baseline: 1.8351x; 1.8351x over previous
"""Bahdanau attention TRN2 Bass kernel (SPMD over 8 NeuronCores).

Problem: S=4096, N=32, H=512 (fp32)
    wh   = hidden @ W_w.T + W_b                      # [N, H]
    ue   = einsum('snh,kh->snk', enc, U_w) + U_b     # [S, N, H]
    en   = tanh(wh + ue)                             # [S, N, H]
    sc   = einsum('snh,oh->sno', en, v_w) + v_b      # [S, N, 1]
    aw   = softmax(sc, axis=0)                       # [S, N, 1]
    ctx  = sum_s aw * enc                            # [N, H]
    returns (ctx, aw)

Sharding: data-parallel over N (4 examples per core), weights replicated.
No cross-core communication.

Single pass over encoder_outputs (the 256 MiB tensor):
  - host feeds enc transposed per-example: encT[n, h, s] (so the TensorE
    contraction dim h lands on SBUF partitions with fast contiguous DMA)
  - ue^T tile [k=128, s=512] = sum_hc U^T_block.T @ encT_tile   (f32r matmuls)
  - energy = tanh(ue^T + (wh+U_b) per-partition bias)           (ACT, fused)
  - score row = sum_k v128.T @ energy  (v replicated across 128 cols so every
    PSUM partition carries the same score row)
  - w = exp(score)  (no max-subtraction needed: |score| <= ||v||_1 ~ 11, and
    exp of that is comfortably inside fp32; softmax is shift-invariant so v_b
    is dropped entirely)  + accum_out gives the Z partial for free
  - ctx partial via one fused DVE tensor_tensor_reduce per (h-chunk, s-block):
    (encT_tile * w) reduced over s -> [128, 1]
  - finals: Z = sum of partials, ctx *= 1/Z, attw row *= 1/Z, DMA out.
"""

import os
from contextlib import ExitStack

import numpy as np

S = 4096
N = 32
H = 512
NCORES = 8
NL = N // NCORES  # 4 examples per core
P = 128
HC = H // P  # 4 h-chunks
KC = H // P  # 4 k-chunks
SBW = 512  # s-block width
SB = S // SBW  # 8 s-blocks

_CACHE = {}


def _build_nc(nl=NL, sb_count=SB):
    import concourse.mybir as mybir
    from concourse.bacc import Bacc
    from concourse.tile import TileContext

    F32 = mybir.dt.float32
    F32R = mybir.dt.float32r
    BF16 = mybir.dt.bfloat16
    AF = mybir.ActivationFunctionType
    ALU = mybir.AluOpType
    AX = mybir.AxisListType

    nc = Bacc(trn_type="TRN2")

    encT = nc.declare_dram_parameter("encT", [NL, H, S], F32R, isOutput=False)
    hidT = nc.declare_dram_parameter("hidT", [H, NL], F32R, isOutput=False)
    WT = nc.declare_dram_parameter("WT", [H, H], F32R, isOutput=False)
    UT = nc.declare_dram_parameter("UT", [H, H], F32R, isOutput=False)
    wb2 = nc.declare_dram_parameter("wb2", [P, KC], F32, isOutput=False)
    ub2 = nc.declare_dram_parameter("ub2", [P, KC], F32, isOutput=False)
    v2 = nc.declare_dram_parameter("v2", [P, KC], F32, isOutput=False)
    ctx_out = nc.declare_dram_parameter("ctx", [NL, H], F32, isOutput=True)
    attw_out = nc.declare_dram_parameter("attw", [NL, S], F32, isOutput=True)

    with TileContext(nc) as tc, ExitStack() as ctx:
        consts = ctx.enter_context(tc.tile_pool(name="consts", bufs=1))
        enc_pool = ctx.enter_context(tc.tile_pool(name="enc", bufs=3))
        en_pool = ctx.enter_context(tc.tile_pool(name="en", bufs=4))
        w_pool = ctx.enter_context(tc.tile_pool(name="w", bufs=3))
        fin_pool = ctx.enter_context(tc.tile_pool(name="fin", bufs=2))
        scr_pool = ctx.enter_context(tc.tile_pool(name="scr", bufs=2))
        psum_ue = ctx.enter_context(tc.tile_pool(name="psue", bufs=4, space="PSUM"))
        psum_sc = ctx.enter_context(tc.tile_pool(name="pssc", bufs=2, space="PSUM"))

        # ---- constants ----
        ut_sb = consts.tile([P, HC, H], F32R)
        UTr = UT.rearrange("(c p) k -> p c k", p=P)
        for hc in range(HC):
            nc.sync.dma_start(out=ut_sb[:, hc, :], in_=UTr[:, hc, :])
        wt_sb = consts.tile([P, HC, H], F32R)
        nc.sync.dma_start(out=wt_sb, in_=WT.rearrange("(c p) k -> p c k", p=P))
        hid_sb = consts.tile([P, HC, NL], F32R)
        nc.sync.dma_start(out=hid_sb, in_=hidT.rearrange("(c p) n -> p c n", p=P))
        wb2_sb = consts.tile([P, KC], F32)
        nc.sync.dma_start(out=wb2_sb, in_=wb2[:])
        ub2_sb = consts.tile([P, KC], F32)
        nc.sync.dma_start(out=ub2_sb, in_=ub2[:])
        v2_sb = consts.tile([P, KC], F32)
        nc.sync.dma_start(out=v2_sb, in_=v2[:])

        bias2 = consts.tile([P, KC], F32)
        nc.vector.tensor_add(bias2, wb2_sb, ub2_sb)

        ones128 = consts.tile([P, P], F32)
        nc.vector.memset(ones128, 1.0)
        v128 = consts.tile([P, KC, P], F32R)
        for kc in range(KC):
            nc.vector.tensor_scalar_mul(v128[:, kc, :], ones128, v2_sb[:, kc : kc + 1])

        # ---- wh^T = W @ hidden^T, bias-folded: whb[k, n] = wh^T + W_b + U_b ----
        whb = consts.tile([P, KC, NL], F32)
        for kc in range(KC):
            pw = psum_ue.tile([P, NL], F32, tag="ue")
            for hc in range(HC):
                nc.tensor.matmul(
                    pw,
                    lhsT=wt_sb[:, hc, kc * P : (kc + 1) * P],
                    rhs=hid_sb[:, hc, :],
                    start=(hc == 0),
                    stop=(hc == HC - 1),
                )
            nc.vector.tensor_scalar_add(whb[:, kc, :], pw, bias2[:, kc : kc + 1])

        # ---- accumulators ----
        zp_n = [consts.tile([P, SB // 2], F32, name=f"zp{i}") for i in range(NL)]
        ctxp_n = [consts.tile([P, HC, SB // 2], F32, name=f"ctxp{i}") for i in range(NL)]
        # per-example attention rows: separate [1, S] tiles (partition 0), fed
        # from row 0 of w (all rows of w are identical) — keeps examples
        # dependency-independent and partition-legal
        attw_n = [consts.tile([1, S], F32, name=f"attw{i}") for i in range(NL)]

        # ---- main loop ----
        SB2 = SBW * 2  # 1024-wide s-superblocks: fewer, bigger ACT/DVE/DMA ops
        for n in range(nl):
            encTn = encT[n].rearrange("(c p) s -> p c s", p=P)
            for sb in range(sb_count // 2):
                # one DMA brings all 4 h-chunks of this 1024-wide s-block
                enc4 = enc_pool.tile([P, HC, SB2], F32R, tag="enc")
                nc.sync.dma_start(
                    out=enc4, in_=encTn[:, :, sb * SB2 : (sb + 1) * SB2]
                )
                psc = psum_sc.tile([P, SB2], F32, tag="sc")
                for half in range(2):
                    hs = slice(half * SBW, (half + 1) * SBW)
                    for kc in range(KC):
                        pue = psum_ue.tile([P, SBW], F32, tag="ue")
                        for hc in range(HC):
                            nc.tensor.matmul(
                                pue,
                                lhsT=ut_sb[:, hc, kc * P : (kc + 1) * P],
                                rhs=enc4[:, hc, hs],
                                start=(hc == 0),
                                stop=(hc == HC - 1),
                            )
                        en = en_pool.tile([P, SBW], F32R, tag="en")
                        nc.scalar.activation(
                            out=en,
                            in_=pue,
                            func=AF.Tanh,
                            bias=whb[:, kc, n : n + 1],
                            scale=1.0,
                        )
                        nc.tensor.matmul(
                            psc[:, hs],
                            lhsT=v128[:, kc, :],
                            rhs=en,
                            start=(kc == 0),
                            stop=(kc == KC - 1),
                        )
                w = w_pool.tile([P, SB2], F32, tag="w")
                nc.scalar.activation(
                    out=w,
                    in_=psc,
                    func=AF.Exp,
                    accum_out=zp_n[n][:, sb : sb + 1],
                )
                # stash the (unnormalized) attention row (all rows of w identical)
                nc.vector.tensor_copy(
                    attw_n[n][0:1, sb * SB2 : (sb + 1) * SB2], w[0:1, :]
                )
                scr = scr_pool.tile([P, HC, SB2], F32, tag="scr")
                for hc in range(HC):
                    nc.vector.affine_mul_reduce(
                        out=scr[:, hc, :],
                        accum_out=ctxp_n[n][:, hc, sb : sb + 1],
                        in0=enc4[:, hc, :].bitcast(F32),
                        in1=w,
                        scale=1.0,
                        bias=0.0,
                    )

            # ---- per-example finals (overlap with next n's main loop) ----
            z1 = fin_pool.tile([P, 1], F32, tag="z1")
            nc.vector.reduce_sum(out=z1, in_=zp_n[n], axis=AX.X)
            rz = fin_pool.tile([P, 1], F32, tag="rz")
            nc.vector.reciprocal(out=rz, in_=z1)
            for hc in range(HC):
                c1 = fin_pool.tile([P, 1], F32, tag="c1")
                nc.vector.reduce_sum(out=c1, in_=ctxp_n[n][:, hc, :], axis=AX.X)
                c2 = fin_pool.tile([P, 1], F32, tag="c2")
                nc.vector.tensor_mul(c2, c1, rz)
                nc.sync.dma_start(out=ctx_out[n, hc * P : (hc + 1) * P], in_=c2[:, 0])
            nc.vector.tensor_scalar_mul(attw_n[n], attw_n[n], rz[0:1, 0:1])
            nc.sync.dma_start(out=attw_out[n : n + 1, :], in_=attw_n[n])

    nc.finalize()
    return nc


def _get_nc():
    if "nc" not in _CACHE:
        _CACHE["nc"] = _build_nc()
    return _CACHE["nc"]


def _prep_inputs(hidden, encoder_outputs, W_w, W_b, U_w, U_b, v_w, v_b):
    f = lambda x: np.ascontiguousarray(np.asarray(x, dtype=np.float32))
    hidden = f(hidden)
    W_w, W_b, U_w, U_b, v_w = f(W_w), f(W_b), f(U_w), f(U_b), f(v_w)
    enc = np.asarray(encoder_outputs, dtype=np.float32)

    encT_all = np.ascontiguousarray(enc.transpose(1, 2, 0))  # [N, H, S]
    hidT_all = np.ascontiguousarray(hidden.T)  # [H, N]
    WT = np.ascontiguousarray(W_w.T)
    UT = np.ascontiguousarray(U_w.T)
    wb2 = np.ascontiguousarray(W_b.reshape(KC, P).T)
    ub2 = np.ascontiguousarray(U_b.reshape(KC, P).T)
    v2 = np.ascontiguousarray(v_w.reshape(KC, P).T)

    in_maps = []
    for c in range(NCORES):
        n0 = c * NL
        in_maps.append(
            {
                "encT": encT_all[n0 : n0 + NL],
                "hidT": np.ascontiguousarray(hidT_all[:, n0 : n0 + NL]),
                "WT": WT,
                "UT": UT,
                "wb2": wb2,
                "ub2": ub2,
                "v2": v2,
            }
        )
    return in_maps


def run(trace=False, **inputs):
    """Run the SPMD kernel; returns ((context, attention_weights), raw results)."""
    from concourse.bass_utils import run_bass_kernel_spmd

    in_maps = _prep_inputs(**inputs)
    nc = _get_nc()
    res = run_bass_kernel_spmd(nc, in_maps, core_ids=list(range(NCORES)), trace=trace)
    ctxs = np.concatenate([res.results[i]["ctx"] for i in range(NCORES)], axis=0)
    attws = np.concatenate([res.results[i]["attw"] for i in range(NCORES)], axis=0)
    attention_weights = np.ascontiguousarray(attws.T)[:, :, None]
    return (ctxs, attention_weights), res


def kernel(**inputs):
    out, _ = run(trace=False, **inputs)
    return out


if __name__ == "__main__":
    # quick smoke: build only
    nc = _get_nc()
    print("built ok, instructions:", len(nc.inst_map))


# revision 31
# speedup vs baseline: 1.8680x; 1.0179x over previous
"""Bahdanau attention TRN2 Bass kernel (SPMD over 8 NeuronCores).

Problem: S=4096, N=32, H=512 (fp32)
    wh   = hidden @ W_w.T + W_b                      # [N, H]
    ue   = einsum('snh,kh->snk', enc, U_w) + U_b     # [S, N, H]
    en   = tanh(wh + ue)                             # [S, N, H]
    sc   = einsum('snh,oh->sno', en, v_w) + v_b      # [S, N, 1]
    aw   = softmax(sc, axis=0)                       # [S, N, 1]
    ctx  = sum_s aw * enc                            # [N, H]
    returns (ctx, aw)

Sharding: data-parallel over N (4 examples per core), weights replicated.
No cross-core communication.

Single pass over encoder_outputs (the 256 MiB tensor):
  - host feeds enc transposed per-example: encT[n, h, s] (so the TensorE
    contraction dim h lands on SBUF partitions with fast contiguous DMA)
  - ue^T tile [k=128, s=512] = sum_hc U^T_block.T @ encT_tile   (f32r matmuls)
  - energy = tanh(ue^T + (wh+U_b) per-partition bias)           (ACT, fused)
  - score row = sum_k v128.T @ energy  (v replicated across 128 cols so every
    PSUM partition carries the same score row)
  - w = exp(score)  (no max-subtraction needed: |score| <= ||v||_1 ~ 11, and
    exp of that is comfortably inside fp32; softmax is shift-invariant so v_b
    is dropped entirely)  + accum_out gives the Z partial for free
  - ctx partial via one fused DVE tensor_tensor_reduce per (h-chunk, s-block):
    (encT_tile * w) reduced over s -> [128, 1]
  - finals: Z = sum of partials, ctx *= 1/Z, attw row *= 1/Z, DMA out.
"""

import os
from contextlib import ExitStack

import numpy as np

S = 4096
N = 32
H = 512
NCORES = 8
NL = N // NCORES  # 4 examples per core
P = 128
HC = H // P  # 4 h-chunks
KC = H // P  # 4 k-chunks
SBW = 512  # s-block width
SB = S // SBW  # 8 s-blocks

_CACHE = {}


def _build_nc(nl=NL, sb_count=SB):
    import concourse.mybir as mybir
    from concourse.bacc import Bacc
    from concourse.tile import TileContext

    F32 = mybir.dt.float32
    F32R = mybir.dt.float32r
    BF16 = mybir.dt.bfloat16
    AF = mybir.ActivationFunctionType
    ALU = mybir.AluOpType
    AX = mybir.AxisListType

    nc = Bacc(trn_type="TRN2")

    encT = nc.declare_dram_parameter("encT", [NL, H, S], F32R, isOutput=False)
    hidT = nc.declare_dram_parameter("hidT", [H, NL], F32R, isOutput=False)
    WT = nc.declare_dram_parameter("WT", [H, H], F32R, isOutput=False)
    UTb = nc.declare_dram_parameter("UTb", [H, H], BF16, isOutput=False)
    wb2 = nc.declare_dram_parameter("wb2", [P, KC], F32, isOutput=False)
    ub2 = nc.declare_dram_parameter("ub2", [P, KC], F32, isOutput=False)
    v2 = nc.declare_dram_parameter("v2", [P, KC], F32, isOutput=False)
    ctx_out = nc.declare_dram_parameter("ctx", [NL, H], F32, isOutput=True)
    attw_out = nc.declare_dram_parameter("attw", [NL, S], F32, isOutput=True)

    with TileContext(nc) as tc, ExitStack() as ctx:
        consts = ctx.enter_context(tc.tile_pool(name="consts", bufs=1))
        enc_pool = ctx.enter_context(tc.tile_pool(name="enc", bufs=3))
        en_pool = ctx.enter_context(tc.tile_pool(name="en", bufs=4))
        w_pool = ctx.enter_context(tc.tile_pool(name="w", bufs=3))
        fin_pool = ctx.enter_context(tc.tile_pool(name="fin", bufs=2))
        scr_pool = ctx.enter_context(tc.tile_pool(name="scr", bufs=2))
        encb_pool = ctx.enter_context(tc.tile_pool(name="encb", bufs=3))
        psum_ue = ctx.enter_context(tc.tile_pool(name="psue", bufs=4, space="PSUM"))
        psum_sc = ctx.enter_context(tc.tile_pool(name="pssc", bufs=2, space="PSUM"))

        # ---- constants ----
        ut_sb = consts.tile([P, HC, H], BF16)
        UTr = UTb.rearrange("(c p) k -> p c k", p=P)
        for hc in range(HC):
            nc.sync.dma_start(out=ut_sb[:, hc, :], in_=UTr[:, hc, :])
        wt_sb = consts.tile([P, HC, H], F32R)
        nc.sync.dma_start(out=wt_sb, in_=WT.rearrange("(c p) k -> p c k", p=P))
        hid_sb = consts.tile([P, HC, NL], F32R)
        nc.sync.dma_start(out=hid_sb, in_=hidT.rearrange("(c p) n -> p c n", p=P))
        wb2_sb = consts.tile([P, KC], F32)
        nc.sync.dma_start(out=wb2_sb, in_=wb2[:])
        ub2_sb = consts.tile([P, KC], F32)
        nc.sync.dma_start(out=ub2_sb, in_=ub2[:])
        v2_sb = consts.tile([P, KC], F32)
        nc.sync.dma_start(out=v2_sb, in_=v2[:])

        bias2 = consts.tile([P, KC], F32)
        nc.vector.tensor_add(bias2, wb2_sb, ub2_sb)

        ones128 = consts.tile([P, P], F32)
        nc.vector.memset(ones128, 1.0)
        v128 = consts.tile([P, KC, P], BF16)
        for kc in range(KC):
            nc.vector.tensor_scalar_mul(v128[:, kc, :], ones128, v2_sb[:, kc : kc + 1])

        # ---- wh^T = W @ hidden^T, bias-folded: whb[k, n] = wh^T + W_b + U_b ----
        whb = consts.tile([P, KC, NL], F32)
        for kc in range(KC):
            pw = psum_ue.tile([P, NL], F32, tag="ue")
            for hc in range(HC):
                nc.tensor.matmul(
                    pw,
                    lhsT=wt_sb[:, hc, kc * P : (kc + 1) * P],
                    rhs=hid_sb[:, hc, :],
                    start=(hc == 0),
                    stop=(hc == HC - 1),
                )
            nc.vector.tensor_scalar_add(whb[:, kc, :], pw, bias2[:, kc : kc + 1])

        # ---- accumulators ----
        zp_n = [consts.tile([P, SB // 2], F32, name=f"zp{i}") for i in range(NL)]
        ctxp_n = [consts.tile([P, HC, SB // 2], F32, name=f"ctxp{i}") for i in range(NL)]
        # per-example attention rows: separate [1, S] tiles (partition 0), fed
        # from row 0 of w (all rows of w are identical) — keeps examples
        # dependency-independent and partition-legal
        attw_n = [consts.tile([1, S], F32, name=f"attw{i}") for i in range(NL)]

        # ---- main loop ----
        SB2 = SBW * 2  # 1024-wide s-superblocks: fewer, bigger ACT/DVE/DMA ops
        for n in range(nl):
            encTn = encT[n].rearrange("(c p) s -> p c s", p=P)
            for sb in range(sb_count // 2):
                # one DMA brings all 4 h-chunks of this 1024-wide s-block
                enc4 = enc_pool.tile([P, HC, SB2], F32R, tag="enc")
                nc.sync.dma_start(
                    out=enc4, in_=encTn[:, :, sb * SB2 : (sb + 1) * SB2]
                )
                enc4b = encb_pool.tile([P, HC, SB2], BF16, tag="encb")
                nc.vector.tensor_copy(enc4b[:, 0:2, :], enc4[:, 0:2, :].bitcast(F32))
                nc.scalar.activation(
                    out=enc4b[:, 2:4, :],
                    in_=enc4[:, 2:4, :].bitcast(F32),
                    func=AF.Copy,
                )
                psc = psum_sc.tile([P, SB2], F32, tag="sc")
                for half in range(2):
                    hs = slice(half * SBW, (half + 1) * SBW)
                    for kc in range(KC):
                        pue = psum_ue.tile([P, SBW], F32, tag="ue")
                        for hc in range(HC):
                            nc.tensor.matmul(
                                pue,
                                lhsT=ut_sb[:, hc, kc * P : (kc + 1) * P],
                                rhs=enc4b[:, hc, hs],
                                start=(hc == 0),
                                stop=(hc == HC - 1),
                            )
                        en = en_pool.tile([P, SBW], BF16, tag="en")
                        nc.scalar.activation(
                            out=en,
                            in_=pue,
                            func=AF.Tanh,
                            bias=whb[:, kc, n : n + 1],
                            scale=1.0,
                        )
                        nc.tensor.matmul(
                            psc[:, hs],
                            lhsT=v128[:, kc, :],
                            rhs=en,
                            start=(kc == 0),
                            stop=(kc == KC - 1),
                        )
                w = w_pool.tile([P, SB2], F32, tag="w")
                nc.scalar.activation(
                    out=w,
                    in_=psc,
                    func=AF.Exp,
                    accum_out=zp_n[n][:, sb : sb + 1],
                )
                # stash the (unnormalized) attention row (all rows of w identical)
                nc.vector.tensor_copy(
                    attw_n[n][0:1, sb * SB2 : (sb + 1) * SB2], w[0:1, :]
                )
                scr = scr_pool.tile([P, HC, SB2], F32, tag="scr")
                for hc in range(HC):
                    nc.vector.affine_mul_reduce(
                        out=scr[:, hc, :],
                        accum_out=ctxp_n[n][:, hc, sb : sb + 1],
                        in0=enc4[:, hc, :].bitcast(F32),
                        in1=w,
                        scale=1.0,
                        bias=0.0,
                    )

            # ---- per-example finals (overlap with next n's main loop) ----
            z1 = fin_pool.tile([P, 1], F32, tag="z1")
            nc.vector.reduce_sum(out=z1, in_=zp_n[n], axis=AX.X)
            rz = fin_pool.tile([P, 1], F32, tag="rz")
            nc.vector.reciprocal(out=rz, in_=z1)
            for hc in range(HC):
                c1 = fin_pool.tile([P, 1], F32, tag="c1")
                nc.vector.reduce_sum(out=c1, in_=ctxp_n[n][:, hc, :], axis=AX.X)
                c2 = fin_pool.tile([P, 1], F32, tag="c2")
                nc.vector.tensor_mul(c2, c1, rz)
                nc.sync.dma_start(out=ctx_out[n, hc * P : (hc + 1) * P], in_=c2[:, 0])
            nc.vector.tensor_scalar_mul(attw_n[n], attw_n[n], rz[0:1, 0:1])
            nc.sync.dma_start(out=attw_out[n : n + 1, :], in_=attw_n[n])

    nc.finalize()
    return nc


def _get_nc():
    if "nc" not in _CACHE:
        _CACHE["nc"] = _build_nc()
    return _CACHE["nc"]


def _prep_inputs(hidden, encoder_outputs, W_w, W_b, U_w, U_b, v_w, v_b):
    f = lambda x: np.ascontiguousarray(np.asarray(x, dtype=np.float32))
    hidden = f(hidden)
    W_w, W_b, U_w, U_b, v_w = f(W_w), f(W_b), f(U_w), f(U_b), f(v_w)
    enc = np.asarray(encoder_outputs, dtype=np.float32)

    encT_all = np.ascontiguousarray(enc.transpose(1, 2, 0))  # [N, H, S]
    hidT_all = np.ascontiguousarray(hidden.T)  # [H, N]
    WT = np.ascontiguousarray(W_w.T)
    import ml_dtypes
    UTb = np.ascontiguousarray(U_w.T).astype(ml_dtypes.bfloat16)
    wb2 = np.ascontiguousarray(W_b.reshape(KC, P).T)
    ub2 = np.ascontiguousarray(U_b.reshape(KC, P).T)
    v2 = np.ascontiguousarray(v_w.reshape(KC, P).T)

    in_maps = []
    for c in range(NCORES):
        n0 = c * NL
        in_maps.append(
            {
                "encT": encT_all[n0 : n0 + NL],
                "hidT": np.ascontiguousarray(hidT_all[:, n0 : n0 + NL]),
                "WT": WT,
                "UTb": UTb,
                "wb2": wb2,
                "ub2": ub2,
                "v2": v2,
            }
        )
    return in_maps


def run(trace=False, **inputs):
    """Run the SPMD kernel; returns ((context, attention_weights), raw results)."""
    from concourse.bass_utils import run_bass_kernel_spmd

    in_maps = _prep_inputs(**inputs)
    nc = _get_nc()
    res = run_bass_kernel_spmd(nc, in_maps, core_ids=list(range(NCORES)), trace=trace)
    ctxs = np.concatenate([res.results[i]["ctx"] for i in range(NCORES)], axis=0)
    attws = np.concatenate([res.results[i]["attw"] for i in range(NCORES)], axis=0)
    attention_weights = np.ascontiguousarray(attws.T)[:, :, None]
    return (ctxs, attention_weights), res


def kernel(**inputs):
    out, _ = run(trace=False, **inputs)
    return out


if __name__ == "__main__":
    # quick smoke: build only
    nc = _get_nc()
    print("built ok, instructions:", len(nc.inst_map))


# revision 32
# speedup vs baseline: 1.8776x; 1.0052x over previous
"""Bahdanau attention TRN2 Bass kernel (SPMD over 8 NeuronCores).

Problem: S=4096, N=32, H=512 (fp32)
    wh   = hidden @ W_w.T + W_b                      # [N, H]
    ue   = einsum('snh,kh->snk', enc, U_w) + U_b     # [S, N, H]
    en   = tanh(wh + ue)                             # [S, N, H]
    sc   = einsum('snh,oh->sno', en, v_w) + v_b      # [S, N, 1]
    aw   = softmax(sc, axis=0)                       # [S, N, 1]
    ctx  = sum_s aw * enc                            # [N, H]
    returns (ctx, aw)

Sharding: data-parallel over N (4 examples per core), weights replicated.
No cross-core communication.

Single pass over encoder_outputs (the 256 MiB tensor):
  - host feeds enc transposed per-example: encT[n, h, s] (so the TensorE
    contraction dim h lands on SBUF partitions with fast contiguous DMA)
  - ue^T tile [k=128, s=1024] = sum_hc U^T_block.T @ encT_tile  (f32r matmuls,
    full fp32 range, ~1.4e-4 rel err)
  - energy = tanh(ue^T + (wh+U_b) per-partition bias)           (ACT, fused)
  - score row = sum_k v128.T @ energy  (v replicated across 128 cols so every
    PSUM partition carries the same score row)
  - w = exp(score)  (no max-subtraction needed: |score| <= ||v||_1 ~ 11, and
    exp of that is comfortably inside fp32; softmax is shift-invariant so v_b
    is dropped entirely)  + accum_out gives the Z partial for free
  - ctx partial via one fused DVE affine_mul_reduce per (h-chunk, s-block):
    (encT_tile * w) reduced over s -> [128, 1], full fp32 enc precision
  - finals: Z = sum of partials, ctx *= 1/Z, attw row *= 1/Z, DMA out.
"""

import os
from contextlib import ExitStack

import numpy as np

S = 4096
N = 32
H = 512
NCORES = 8
NL = N // NCORES  # 4 examples per core
P = 128
HC = H // P  # 4 h-chunks
KC = H // P  # 4 k-chunks
SBW = 512  # s-block width
SB = S // SBW  # 8 s-blocks

_CACHE = {}


def _build_nc(nl=NL, sb_count=SB):
    import concourse.mybir as mybir
    from concourse.bacc import Bacc
    from concourse.tile import TileContext

    F32 = mybir.dt.float32
    F32R = mybir.dt.float32r
    BF16 = mybir.dt.bfloat16
    AF = mybir.ActivationFunctionType
    ALU = mybir.AluOpType
    AX = mybir.AxisListType

    nc = Bacc(trn_type="TRN2")

    encT = nc.declare_dram_parameter("encT", [NL, H, S], F32R, isOutput=False)
    hidT = nc.declare_dram_parameter("hidT", [H, NL], F32R, isOutput=False)
    WT = nc.declare_dram_parameter("WT", [H, H], F32R, isOutput=False)
    UT = nc.declare_dram_parameter("UT", [H, H], F32R, isOutput=False)
    wb2 = nc.declare_dram_parameter("wb2", [P, KC], F32, isOutput=False)
    ub2 = nc.declare_dram_parameter("ub2", [P, KC], F32, isOutput=False)
    v2 = nc.declare_dram_parameter("v2", [P, KC], F32, isOutput=False)
    ctx_out = nc.declare_dram_parameter("ctx", [NL, H], F32, isOutput=True)
    attw_out = nc.declare_dram_parameter("attw", [NL, S], F32, isOutput=True)

    with TileContext(nc) as tc, ExitStack() as ctx:
        consts = ctx.enter_context(tc.tile_pool(name="consts", bufs=1))
        enc_pool = ctx.enter_context(tc.tile_pool(name="enc", bufs=3))
        en_pool = ctx.enter_context(tc.tile_pool(name="en", bufs=4))
        w_pool = ctx.enter_context(tc.tile_pool(name="w", bufs=3))
        fin_pool = ctx.enter_context(tc.tile_pool(name="fin", bufs=2))
        scr_pool = ctx.enter_context(tc.tile_pool(name="scr", bufs=2))
        psum_ue = ctx.enter_context(tc.tile_pool(name="psue", bufs=4, space="PSUM"))
        psum_sc = ctx.enter_context(tc.tile_pool(name="pssc", bufs=2, space="PSUM"))

        # ---- constants ----
        ut_sb = consts.tile([P, HC, H], F32R)
        UTr = UT.rearrange("(c p) k -> p c k", p=P)
        for hc in range(HC):
            nc.sync.dma_start(out=ut_sb[:, hc, :], in_=UTr[:, hc, :])
        wt_sb = consts.tile([P, HC, H], F32R)
        nc.sync.dma_start(out=wt_sb, in_=WT.rearrange("(c p) k -> p c k", p=P))
        hid_sb = consts.tile([P, HC, NL], F32R)
        nc.sync.dma_start(out=hid_sb, in_=hidT.rearrange("(c p) n -> p c n", p=P))
        wb2_sb = consts.tile([P, KC], F32)
        nc.sync.dma_start(out=wb2_sb, in_=wb2[:])
        ub2_sb = consts.tile([P, KC], F32)
        nc.sync.dma_start(out=ub2_sb, in_=ub2[:])
        v2_sb = consts.tile([P, KC], F32)
        nc.sync.dma_start(out=v2_sb, in_=v2[:])

        bias2 = consts.tile([P, KC], F32)
        nc.vector.tensor_add(bias2, wb2_sb, ub2_sb)

        ones128 = consts.tile([P, P], F32)
        nc.vector.memset(ones128, 1.0)
        v128 = consts.tile([P, KC, P], F32R)
        for kc in range(KC):
            nc.vector.tensor_scalar_mul(v128[:, kc, :], ones128, v2_sb[:, kc : kc + 1])

        # ---- wh^T = W @ hidden^T, bias-folded: whb[k, n] = wh^T + W_b + U_b ----
        whb = consts.tile([P, KC, NL], F32)
        for kc in range(KC):
            pw = psum_ue.tile([P, NL], F32, tag="ue")
            for hc in range(HC):
                nc.tensor.matmul(
                    pw,
                    lhsT=wt_sb[:, hc, kc * P : (kc + 1) * P],
                    rhs=hid_sb[:, hc, :],
                    start=(hc == 0),
                    stop=(hc == HC - 1),
                )
            nc.vector.tensor_scalar_add(whb[:, kc, :], pw, bias2[:, kc : kc + 1])

        # ---- accumulators ----
        zp_n = [consts.tile([P, SB // 2], F32, name=f"zp{i}") for i in range(NL)]
        ctxp_n = [consts.tile([P, HC, SB // 2], F32, name=f"ctxp{i}") for i in range(NL)]
        # per-example attention rows: separate [1, S] tiles (partition 0), fed
        # from row 0 of w (all rows of w are identical) — keeps examples
        # dependency-independent and partition-legal
        attw_n = [consts.tile([1, S], F32, name=f"attw{i}") for i in range(NL)]

        # ---- main loop ----
        SB2 = SBW * 2  # 1024-wide s-superblocks: fewer, bigger ACT/DVE/DMA ops
        for n in range(nl):
            encTn = encT[n].rearrange("(c p) s -> p c s", p=P)
            for sb in range(sb_count // 2):
                # one DMA brings all 4 h-chunks of this 1024-wide s-block
                enc4 = enc_pool.tile([P, HC, SB2], F32R, tag="enc")
                nc.sync.dma_start(
                    out=enc4, in_=encTn[:, :, sb * SB2 : (sb + 1) * SB2]
                )
                psc = psum_sc.tile([P, SB2], F32, tag="sc")
                for half in range(2):
                    hs = slice(half * SBW, (half + 1) * SBW)
                    for kc in range(KC):
                        pue = psum_ue.tile([P, SBW], F32, tag="ue")
                        for hc in range(HC):
                            nc.tensor.matmul(
                                pue,
                                lhsT=ut_sb[:, hc, kc * P : (kc + 1) * P],
                                rhs=enc4[:, hc, hs],
                                start=(hc == 0),
                                stop=(hc == HC - 1),
                            )
                        en = en_pool.tile([P, SBW], F32R, tag="en")
                        nc.scalar.activation(
                            out=en,
                            in_=pue,
                            func=AF.Tanh,
                            bias=whb[:, kc, n : n + 1],
                            scale=1.0,
                        )
                        nc.tensor.matmul(
                            psc[:, hs],
                            lhsT=v128[:, kc, :],
                            rhs=en,
                            start=(kc == 0),
                            stop=(kc == KC - 1),
                        )
                w = w_pool.tile([P, SB2], F32, tag="w")
                nc.scalar.activation(
                    out=w,
                    in_=psc,
                    func=AF.Exp,
                    accum_out=zp_n[n][:, sb : sb + 1],
                )
                # stash the (unnormalized) attention row (all rows of w identical)
                nc.vector.tensor_copy(
                    attw_n[n][0:1, sb * SB2 : (sb + 1) * SB2], w[0:1, :]
                )
                scr = scr_pool.tile([P, HC, SB2], F32, tag="scr")
                for hc in range(HC):
                    nc.vector.affine_mul_reduce(
                        out=scr[:, hc, :],
                        accum_out=ctxp_n[n][:, hc, sb : sb + 1],
                        in0=enc4[:, hc, :].bitcast(F32),
                        in1=w,
                        scale=1.0,
                        bias=0.0,
                    )

            # ---- per-example finals (overlap with next n's main loop) ----
            z1 = fin_pool.tile([P, 1], F32, tag="z1")
            nc.vector.reduce_sum(out=z1, in_=zp_n[n], axis=AX.X)
            rz = fin_pool.tile([P, 1], F32, tag="rz")
            nc.vector.reciprocal(out=rz, in_=z1)
            for hc in range(HC):
                c1 = fin_pool.tile([P, 1], F32, tag="c1")
                nc.vector.reduce_sum(out=c1, in_=ctxp_n[n][:, hc, :], axis=AX.X)
                c2 = fin_pool.tile([P, 1], F32, tag="c2")
                nc.vector.tensor_mul(c2, c1, rz)
                nc.sync.dma_start(out=ctx_out[n, hc * P : (hc + 1) * P], in_=c2[:, 0])
            nc.vector.tensor_scalar_mul(attw_n[n], attw_n[n], rz[0:1, 0:1])
            nc.sync.dma_start(out=attw_out[n : n + 1, :], in_=attw_n[n])

    nc.finalize()
    return nc


def _get_nc():
    if "nc" not in _CACHE:
        _CACHE["nc"] = _build_nc()
    return _CACHE["nc"]


def _prep_inputs(hidden, encoder_outputs, W_w, W_b, U_w, U_b, v_w, v_b):
    f = lambda x: np.ascontiguousarray(np.asarray(x, dtype=np.float32))
    hidden = f(hidden)
    W_w, W_b, U_w, U_b, v_w = f(W_w), f(W_b), f(U_w), f(U_b), f(v_w)
    enc = np.asarray(encoder_outputs, dtype=np.float32)

    encT_all = np.ascontiguousarray(enc.transpose(1, 2, 0))  # [N, H, S]
    hidT_all = np.ascontiguousarray(hidden.T)  # [H, N]
    WT = np.ascontiguousarray(W_w.T)
    UT = np.ascontiguousarray(U_w.T)
    wb2 = np.ascontiguousarray(W_b.reshape(KC, P).T)
    ub2 = np.ascontiguousarray(U_b.reshape(KC, P).T)
    v2 = np.ascontiguousarray(v_w.reshape(KC, P).T)

    in_maps = []
    for c in range(NCORES):
        n0 = c * NL
        in_maps.append(
            {
                "encT": encT_all[n0 : n0 + NL],
                "hidT": np.ascontiguousarray(hidT_all[:, n0 : n0 + NL]),
                "WT": WT,
                "UT": UT,
                "wb2": wb2,
                "ub2": ub2,
                "v2": v2,
            }
        )
    return in_maps


def run(trace=False, **inputs):
    """Run the SPMD kernel; returns ((context, attention_weights), raw results)."""
    from concourse.bass_utils import run_bass_kernel_spmd

    in_maps = _prep_inputs(**inputs)
    nc = _get_nc()
    res = run_bass_kernel_spmd(nc, in_maps, core_ids=list(range(NCORES)), trace=trace)
    ctxs = np.concatenate([res.results[i]["ctx"] for i in range(NCORES)], axis=0)
    attws = np.concatenate([res.results[i]["attw"] for i in range(NCORES)], axis=0)
    attention_weights = np.ascontiguousarray(attws.T)[:, :, None]
    return (ctxs, attention_weights), res


def kernel(**inputs):
    out, _ = run(trace=False, **inputs)
    return out


if __name__ == "__main__":
    # quick smoke: build only
    nc = _get_nc()
    print("built ok, instructions:", len(nc.inst_map))


# revision 34
# speedup vs baseline: 2.1604x; 1.1506x over previous
"""Bahdanau attention TRN2 Bass kernel (SPMD over 8 NeuronCores).

Problem: S=4096, N=32, H=512 (fp32)
    wh   = hidden @ W_w.T + W_b                      # [N, H]
    ue   = einsum('snh,kh->snk', enc, U_w) + U_b     # [S, N, H]
    en   = tanh(wh + ue)                             # [S, N, H]
    sc   = einsum('snh,oh->sno', en, v_w) + v_b      # [S, N, 1]
    aw   = softmax(sc, axis=0)                       # [S, N, 1]
    ctx  = sum_s aw * enc                            # [N, H]
    returns (ctx, aw)

Sharding: data-parallel over N (4 examples per core), weights replicated.
No cross-core communication.

Single pass over encoder_outputs (the 256 MiB tensor):
  - host feeds enc transposed per-example: encT[n, h, s] (so the TensorE
    contraction dim h lands on SBUF partitions with fast contiguous DMA)
  - ue^T tile [k=128, s=1024] = sum_hc U^T_block.T @ encT_tile  (f32r matmuls,
    full fp32 range, ~1.4e-4 rel err)
  - energy = tanh(ue^T + (wh+U_b) per-partition bias)           (ACT, fused)
  - score row = sum_k v128.T @ energy  (v replicated across 128 cols so every
    PSUM partition carries the same score row)
  - w = exp(score)  (no max-subtraction needed: |score| <= ||v||_1 ~ 11, and
    exp of that is comfortably inside fp32; softmax is shift-invariant so v_b
    is dropped entirely)  + accum_out gives the Z partial for free
  - ctx partial via one fused DVE affine_mul_reduce per (h-chunk, s-block):
    (encT_tile * w) reduced over s -> [128, 1], full fp32 enc precision
  - finals: Z = sum of partials, ctx *= 1/Z, attw row *= 1/Z, DMA out.
"""

import os
from contextlib import ExitStack

import numpy as np

S = 4096
N = 32
H = 512
NCORES = 8
NL = N // NCORES  # 4 examples per core
P = 128
HC = H // P  # 4 h-chunks
KC = H // P  # 4 k-chunks
SBW = 512  # s-block width
SB = S // SBW  # 8 s-blocks

_CACHE = {}


def _build_nc(nl=NL, sb_count=SB):
    import concourse.mybir as mybir
    from concourse.bacc import Bacc
    from concourse.tile import TileContext

    F32 = mybir.dt.float32
    F32R = mybir.dt.float32r
    BF16 = mybir.dt.bfloat16
    AF = mybir.ActivationFunctionType
    ALU = mybir.AluOpType
    AX = mybir.AxisListType

    nc = Bacc(trn_type="TRN2")

    encT = nc.declare_dram_parameter("encT", [NL, H, S], F32R, isOutput=False)
    hidT = nc.declare_dram_parameter("hidT", [H, NL], F32R, isOutput=False)
    WT = nc.declare_dram_parameter("WT", [H, H], F32R, isOutput=False)
    UT = nc.declare_dram_parameter("UT", [H, H], F32R, isOutput=False)
    wb2 = nc.declare_dram_parameter("wb2", [P, KC], F32, isOutput=False)
    ub2 = nc.declare_dram_parameter("ub2", [P, KC], F32, isOutput=False)
    v2 = nc.declare_dram_parameter("v2", [P, KC], F32, isOutput=False)
    ctx_out = nc.declare_dram_parameter("ctx", [NL, H], F32, isOutput=True)
    attw_out = nc.declare_dram_parameter("attw", [NL, S], F32, isOutput=True)

    with TileContext(nc) as tc, ExitStack() as ctx:
        consts = ctx.enter_context(tc.tile_pool(name="consts", bufs=1))
        enc_pool = ctx.enter_context(tc.tile_pool(name="enc", bufs=4))
        en_pool = ctx.enter_context(tc.tile_pool(name="en", bufs=4))
        w_pool = ctx.enter_context(tc.tile_pool(name="w", bufs=3))
        fin_pool = ctx.enter_context(tc.tile_pool(name="fin", bufs=2))
        scr_pool = ctx.enter_context(tc.tile_pool(name="scr", bufs=2))
        psum_ue = ctx.enter_context(tc.tile_pool(name="psue", bufs=4, space="PSUM"))
        psum_sc = ctx.enter_context(tc.tile_pool(name="pssc", bufs=2, space="PSUM"))

        # ---- constants ----
        ut_sb = consts.tile([P, HC, H], F32R)
        UTr = UT.rearrange("(c p) k -> p c k", p=P)
        for hc in range(HC):
            nc.sync.dma_start(out=ut_sb[:, hc, :], in_=UTr[:, hc, :])
        wt_sb = consts.tile([P, HC, H], F32R)
        nc.sync.dma_start(out=wt_sb, in_=WT.rearrange("(c p) k -> p c k", p=P))
        hid_sb = consts.tile([P, HC, NL], F32R)
        nc.sync.dma_start(out=hid_sb, in_=hidT.rearrange("(c p) n -> p c n", p=P))
        wb2_sb = consts.tile([P, KC], F32)
        nc.sync.dma_start(out=wb2_sb, in_=wb2[:])
        ub2_sb = consts.tile([P, KC], F32)
        nc.sync.dma_start(out=ub2_sb, in_=ub2[:])
        v2_sb = consts.tile([P, KC], F32)
        nc.sync.dma_start(out=v2_sb, in_=v2[:])

        bias2 = consts.tile([P, KC], F32)
        nc.vector.tensor_add(bias2, wb2_sb, ub2_sb)

        ones128 = consts.tile([P, P], F32)
        nc.vector.memset(ones128, 1.0)
        v128 = consts.tile([P, KC, P], F32R)
        for kc in range(KC):
            nc.vector.tensor_scalar_mul(v128[:, kc, :], ones128, v2_sb[:, kc : kc + 1])

        # ---- wh^T = W @ hidden^T, bias-folded: whb[k, n] = wh^T + W_b + U_b ----
        whb = consts.tile([P, KC, NL], F32)
        for kc in range(KC):
            pw = psum_ue.tile([P, NL], F32, tag="ue")
            for hc in range(HC):
                nc.tensor.matmul(
                    pw,
                    lhsT=wt_sb[:, hc, kc * P : (kc + 1) * P],
                    rhs=hid_sb[:, hc, :],
                    start=(hc == 0),
                    stop=(hc == HC - 1),
                )
            nc.vector.tensor_scalar_add(whb[:, kc, :], pw, bias2[:, kc : kc + 1])

        # ---- accumulators ----
        zp_n = [consts.tile([P, SB // 2], F32, name=f"zp{i}") for i in range(NL)]
        ctxp_n = [consts.tile([P, HC, SB // 2], F32, name=f"ctxp{i}") for i in range(NL)]
        # per-example attention rows: separate [1, S] tiles (partition 0), fed
        # from row 0 of w (all rows of w are identical) — keeps examples
        # dependency-independent and partition-legal
        attw_n = [consts.tile([1, S], F32, name=f"attw{i}") for i in range(NL)]

        # ---- per-example finals, emitted deferred (one example late) so the
        # finals chain fills pipeline bubbles instead of stalling the DVE FIFO
        def emit_finals(n):
            z1 = fin_pool.tile([P, 1], F32, tag="z1")
            nc.vector.reduce_sum(out=z1, in_=zp_n[n], axis=AX.X)
            rz = fin_pool.tile([P, 1], F32, tag="rz")
            nc.vector.reciprocal(out=rz, in_=z1)
            for hc in range(HC):
                c1 = fin_pool.tile([P, 1], F32, tag="c1")
                nc.vector.reduce_sum(out=c1, in_=ctxp_n[n][:, hc, :], axis=AX.X)
                c2 = fin_pool.tile([P, 1], F32, tag="c2")
                nc.vector.tensor_mul(c2, c1, rz)
                nc.sync.dma_start(out=ctx_out[n, hc * P : (hc + 1) * P], in_=c2[:, 0])
            nc.vector.tensor_scalar_mul(attw_n[n], attw_n[n], rz[0:1, 0:1])
            nc.sync.dma_start(out=attw_out[n : n + 1, :], in_=attw_n[n])

        # ---- main loop ----
        SB2 = SBW * 2  # 1024-wide s-superblocks: fewer, bigger ACT/DVE/DMA ops
        for n in range(nl):
            encTn = encT[n].rearrange("(c p) s -> p c s", p=P)
            for sb in range(sb_count // 2):
                # one DMA brings all 4 h-chunks of this 1024-wide s-block
                enc4 = enc_pool.tile([P, HC, SB2], F32R, tag="enc")
                nc.sync.dma_start(
                    out=enc4, in_=encTn[:, :, sb * SB2 : (sb + 1) * SB2]
                )
                psc = psum_sc.tile([P, SB2], F32, tag="sc")
                for half in range(2):
                    hs = slice(half * SBW, (half + 1) * SBW)
                    for kc in range(KC):
                        pue = psum_ue.tile([P, SBW], F32, tag="ue")
                        for hc in range(HC):
                            nc.tensor.matmul(
                                pue,
                                lhsT=ut_sb[:, hc, kc * P : (kc + 1) * P],
                                rhs=enc4[:, hc, hs],
                                start=(hc == 0),
                                stop=(hc == HC - 1),
                            )
                        en = en_pool.tile([P, SBW], F32R, tag="en")
                        nc.scalar.activation(
                            out=en,
                            in_=pue,
                            func=AF.Tanh,
                            bias=whb[:, kc, n : n + 1],
                            scale=1.0,
                        )
                        nc.tensor.matmul(
                            psc[:, hs],
                            lhsT=v128[:, kc, :],
                            rhs=en,
                            start=(kc == 0),
                            stop=(kc == KC - 1),
                        )
                w = w_pool.tile([P, SB2], F32, tag="w")
                nc.scalar.activation(
                    out=w,
                    in_=psc,
                    func=AF.Exp,
                    accum_out=zp_n[n][:, sb : sb + 1],
                )
                # stash the (unnormalized) attention row (all rows of w identical)
                nc.vector.tensor_copy(
                    attw_n[n][0:1, sb * SB2 : (sb + 1) * SB2], w[0:1, :]
                )
                scr = scr_pool.tile([P, HC, SB2], F32, tag="scr")
                for hc in range(HC):
                    nc.vector.affine_mul_reduce(
                        out=scr[:, hc, :],
                        accum_out=ctxp_n[n][:, hc, sb : sb + 1],
                        in0=enc4[:, hc, :].bitcast(F32),
                        in1=w,
                        scale=1.0,
                        bias=0.0,
                    )
                if n > 0 and sb == 1:
                    emit_finals(n - 1)
        emit_finals(nl - 1)

    nc.finalize()
    return nc


def _get_nc():
    if "nc" not in _CACHE:
        _CACHE["nc"] = _build_nc()
    return _CACHE["nc"]


def _prep_inputs(hidden, encoder_outputs, W_w, W_b, U_w, U_b, v_w, v_b):
    f = lambda x: np.ascontiguousarray(np.asarray(x, dtype=np.float32))
    hidden = f(hidden)
    W_w, W_b, U_w, U_b, v_w = f(W_w), f(W_b), f(U_w), f(U_b), f(v_w)
    enc = np.asarray(encoder_outputs, dtype=np.float32)

    encT_all = np.ascontiguousarray(enc.transpose(1, 2, 0))  # [N, H, S]
    hidT_all = np.ascontiguousarray(hidden.T)  # [H, N]
    WT = np.ascontiguousarray(W_w.T)
    UT = np.ascontiguousarray(U_w.T)
    wb2 = np.ascontiguousarray(W_b.reshape(KC, P).T)
    ub2 = np.ascontiguousarray(U_b.reshape(KC, P).T)
    v2 = np.ascontiguousarray(v_w.reshape(KC, P).T)

    in_maps = []
    for c in range(NCORES):
        n0 = c * NL
        in_maps.append(
            {
                "encT": encT_all[n0 : n0 + NL],
                "hidT": np.ascontiguousarray(hidT_all[:, n0 : n0 + NL]),
                "WT": WT,
                "UT": UT,
                "wb2": wb2,
                "ub2": ub2,
                "v2": v2,
            }
        )
    return in_maps


def run(trace=False, **inputs):
    """Run the SPMD kernel; returns ((context, attention_weights), raw results)."""
    from concourse.bass_utils import run_bass_kernel_spmd

    in_maps = _prep_inputs(**inputs)
    nc = _get_nc()
    res = run_bass_kernel_spmd(nc, in_maps, core_ids=list(range(NCORES)), trace=trace)
    ctxs = np.concatenate([res.results[i]["ctx"] for i in range(NCORES)], axis=0)
    attws = np.concatenate([res.results[i]["attw"] for i in range(NCORES)], axis=0)
    attention_weights = np.ascontiguousarray(attws.T)[:, :, None]
    return (ctxs, attention_weights), res


def kernel(**inputs):
    out, _ = run(trace=False, **inputs)
    return out


if __name__ == "__main__":
    # quick smoke: build only
    nc = _get_nc()
    print("built ok, instructions:", len(nc.inst_map))
